# revision 7
# baseline (speedup 1.0000x reference)
# Mistral-style GQA attention layer (QKV proj + RoPE + causal attention +
# o_proj), tensor-parallel over heads across 8 NeuronCores.
#
# Sharding (8-way TP over heads): core c owns q heads [4c..4c+4) and kv head c.
#   - w_qkv rows sharded: 4 q-head blocks + 1 k block + 1 v block per core
#   - w_o columns sharded: each core computes a partial o_proj output,
#     host sums the 8 partials (the "all-reduce").
#
# Device kernel (identical SPMD program, per-core data):
#   phase 1: qkvT = Wc @ X^T (outputs TRANSPOSED: [dim, t]) + inline RoPE on
#            q/k rows, spilled to DRAM scratch.
#   phase 2: per (batch, head): S^T = K Q^T on PE, exp on ACT (no max-sub:
#            scores are O(5) and fp32 exp is safe), causal mask via
#            multiplicative 0/1 tiles on DVE, softmax denominator via
#            ones-matmul over the partition (key) axis, P@V with V as the
#            stationary operand (needs V natural layout -> 128x128 PE
#            transposes of V^T), normalize at the end.
#   phase 3: o_partial = attn @ Wo_c^T using attn^T tiles as lhsT.
#
# All matmuls run as float32r (FP22 multiply, fp32 accumulate): full PE rate
# at moving-dim >= 256, ~1e-4 relative error.

import os
from dataclasses import dataclass

import numpy as np

import concourse.bass as bass
from concourse import bacc
import concourse.mybir as mybir
import concourse.tile as tile
from concourse.bass import ds, ts
from concourse.bass_utils import run_bass_kernel_spmd
from concourse.masks import make_identity

F32 = mybir.dt.float32
F32R = mybir.dt.float32r
EXPF = mybir.ActivationFunctionType.Exp


@dataclass(frozen=True)
class Cfg:
    T: int = 4096          # total tokens (B*S)
    H: int = 4096          # hidden size
    S: int = 1024          # seq len
    nq: int = 4            # q heads per core
    n_cores: int = 8
    D: int = 128           # head dim

    @property
    def B(self):
        return self.T // self.S

    @property
    def KO(self):  # contraction tiles for qkv proj
        return self.H // 128

    @property
    def NM(self):  # qkv output row-tiles per core (q heads + k + v)
        return self.nq + 2

    @property
    def QW(self):  # q tile width in attention
        return min(512, self.S)

    @property
    def NJ(self):
        return self.S // self.QW

    @property
    def NKT(self):  # key tiles per batch
        return self.S // 128

    @property
    def NSLAB(self):
        return self.T // 256


FULL = Cfg()


def r32(ap):
    return ap.bitcast(F32R)


def build_nc(cfg: Cfg) -> bass.Bass:  # returns compiled Bacc
    nc = bacc.Bacc("TRN2", target_bir_lowering=False, debug=False, num_devices=cfg.n_cores)
    T, H, S, nq, D = cfg.T, cfg.H, cfg.S, cfg.nq, cfg.D
    KO, NM, QW, NJ, NKT, B = cfg.KO, cfg.NM, cfg.QW, cfg.NJ, cfg.NKT, cfg.B
    NRT = QW // 128  # number of diagonal mask offsets
    scale = 1.0 / np.sqrt(D)

    xT = nc.dram_tensor("xT", [KO, 128, T], F32R, kind="ExternalInput")
    wqkvT = nc.dram_tensor("wqkvT", [KO, 128, NM * 128], F32R, kind="ExternalInput")
    woT = nc.dram_tensor("woT", [nq, 128, H], F32R, kind="ExternalInput")
    cosT = nc.dram_tensor("cosT", [128, S], F32, kind="ExternalInput")
    sinT = nc.dram_tensor("sinT", [128, S], F32, kind="ExternalInput")
    masksT = nc.dram_tensor("masksT", [128, NRT, QW], F32, kind="ExternalInput")
    identT = nc.dram_tensor("identT", [128, 128], F32R, kind="ExternalInput")
    onesT = nc.dram_tensor("onesT", [128, 1], F32R, kind="ExternalInput")
    out = nc.dram_tensor("o_partial", [T, H], F32, kind="ExternalOutput")

    with tile.TileContext(nc) as tc:
        with (
            tc.tile_pool(name="psum", bufs=8, space="PSUM") as psum,
            tc.tile_pool(name="consts", bufs=1) as consts,
            tc.tile_pool(name="dram", bufs=1, space="DRAM") as dram,
        ):
            qkv_sp = dram.tile([NM, 128, T], F32R)

            ident = consts.tile([128, 128], F32R)
            nc.sync.dma_start(ident, identT[:])
            ones = consts.tile([128, 1], F32R)
            nc.sync.dma_start(ones, onesT[:])
            masks_sb = consts.tile([128, NRT, QW], F32)
            nc.sync.dma_start(masks_sb, masksT[:])

            # ---------------- phase 1: QKV projection + RoPE ----------------
            with (
                tc.tile_pool(name="wq", bufs=1) as wq_pool,
                tc.tile_pool(name="xin", bufs=3) as xin,
                tc.tile_pool(name="stage", bufs=2) as stage,
                tc.tile_pool(name="rot", bufs=2) as rot_pool,
                tc.tile_pool(name="tab", bufs=1) as tab,
            ):
                w_all = wq_pool.tile([128, KO, NM * 128], F32R)
                nc.sync.dma_start(w_all, wqkvT[:].rearrange("k p m -> p k m"))
                cos_sb = tab.tile([128, S], F32)
                nc.sync.dma_start(cos_sb, cosT[:])
                sin_sb = tab.tile([128, S], F32)
                nc.sync.dma_start(sin_sb, sinT[:])

                KH = min(16, KO)  # k-tiles per x-slab chunk
                NCH = KO // KH
                for n in range(cfg.NSLAB):
                    tsl = ds(n * 256, 256)
                    xh = []
                    for ch in range(NCH):
                        xt = xin.tile([128, KH, 256], F32R, tag="xh")
                        nc.sync.dma_start(
                            xt, xT[ds(ch * KH, KH), :, tsl].rearrange("k p t -> p k t")
                        )
                        xh.append(xt)
                    st = stage.tile([128, NM, 256], F32R)
                    for m in range(NM):
                        ps = psum.tile([128, 256], F32, tag="bank")
                        for ch in range(NCH):
                            for k in range(KH):
                                nc.tensor.matmul(
                                    ps,
                                    w_all[:, ch * KH + k, ts(m, 128)],
                                    xh[ch][:, k, :],
                                    start=(ch == 0 and k == 0),
                                    stop=(ch == NCH - 1 and k == KH - 1),
                                )
                        nc.scalar.copy(st[:, m, :], ps)
                    # RoPE on q heads + k head (rows 0..nq), not v
                    rot = rot_pool.tile([128, nq + 1, 256], F32R)
                    nc.sync.dma_start(rot[0:64], st[64:128, 0 : nq + 1, :])
                    nc.sync.dma_start(rot[64:128], st[0:64, 0 : nq + 1, :])
                    s0 = (n * 256) % S
                    cos_b = cos_sb[:, None, ds(s0, 256)].to_broadcast((128, nq + 1, 256))
                    sin_b = sin_sb[:, None, ds(s0, 256)].to_broadcast((128, nq + 1, 256))
                    nc.vector.tensor_mul(st[:, 0 : nq + 1, :], st[:, 0 : nq + 1, :], cos_b)
                    nc.vector.tensor_mul(rot, rot, sin_b)
                    nc.vector.tensor_add(st[:, 0 : nq + 1, :], st[:, 0 : nq + 1, :], rot)
                    nc.sync.dma_start(
                        qkv_sp[:, :, tsl].rearrange("m p t -> p m t"), st
                    )

            # ---------------- phase 2: attention ----------------
            with (
                tc.tile_pool(name="attn", bufs=1) as attn_pool,
                tc.tile_pool(name="kv", bufs=2) as kv_pool,
                tc.tile_pool(name="vn", bufs=2) as vn_pool,
                tc.tile_pool(name="qh", bufs=3) as q_pool,
                tc.tile_pool(name="pt", bufs=12) as pt_pool,
                tc.tile_pool(name="sm", bufs=4) as sm_pool,
            ):
                attnT = attn_pool.tile([128, B * nq, S], F32R)
                for b in range(B):
                    bsl = ds(b * S, S)
                    kT = kv_pool.tile([128, S], F32R, tag="kT")
                    nc.sync.dma_start(kT, qkv_sp[nq, :, bsl])
                    vT = kv_pool.tile([128, S], F32R, tag="vT")
                    nc.sync.dma_start(vT, qkv_sp[nq + 1, :, bsl])
                    vn = vn_pool.tile([128, NKT, 128], F32R)
                    for kt in range(NKT):
                        tp = psum.tile([128, 128], F32R, tag="bank")
                        nc.tensor.transpose(tp, vT[:, ts(kt, 128)], ident)
                        nc.vector.tensor_copy(vn[:, kt, :], tp)
                    for h in range(nq):
                        q = q_pool.tile([128, S], F32R)
                        nc.sync.dma_start(q, qkv_sp[h, :, bsl])
                        for j in range(NJ):
                            nkt = (j + 1) * NRT
                            qsl = ds(j * QW, QW)
                            pts = []
                            for kt in range(nkt):
                                sp = psum.tile([128, QW], F32, tag="bank")
                                nc.tensor.matmul(
                                    sp,
                                    kT[:, ts(kt, 128)],
                                    q[:, qsl],
                                    start=True,
                                    stop=True,
                                )
                                pt = pt_pool.tile([128, QW], F32R, tag="pt")
                                nc.scalar.activation(pt, sp, EXPF, scale=scale)
                                if kt >= j * NRT:
                                    nc.vector.tensor_mul(
                                        pt, pt, masks_sb[:, kt - j * NRT, :]
                                    )
                                pts.append(pt)
                            sum_ps = psum.tile([1, QW], F32, tag="bank")
                            for i, pt in enumerate(pts):
                                nc.tensor.matmul(
                                    sum_ps,
                                    ones,
                                    pt,
                                    start=(i == 0),
                                    stop=(i == len(pts) - 1),
                                )
                            recip = sm_pool.tile([1, QW], F32, tag="recip")
                            nc.vector.reciprocal(recip, sum_ps)
                            rb = sm_pool.tile([128, QW], F32, tag="rb")
                            nc.gpsimd.partition_broadcast(rb, recip)
                            pv = psum.tile([128, QW], F32, tag="bank")
                            for i, pt in enumerate(pts):
                                nc.tensor.matmul(
                                    pv,
                                    vn[:, i, :],
                                    pt,
                                    start=(i == 0),
                                    stop=(i == len(pts) - 1),
                                )
                            nc.vector.tensor_mul(
                                attnT[:, b * nq + h, ds(j * QW, QW)], pv, rb
                            )

                # ---------------- phase 3: o_proj ----------------
                with (
                    tc.tile_pool(name="wo", bufs=2) as wo_pool,
                    tc.tile_pool(name="ot", bufs=4) as ot_pool,
                ):
                    for nh in range(H // 512):
                        hsl = ds(nh * 512, 512)
                        wo_t = wo_pool.tile([128, nq, 512], F32R)
                        nc.sync.dma_start(
                            wo_t, woT[:, :, hsl].rearrange("a p n -> p a n")
                        )
                        for tm in range(T // 128):
                            b = (tm * 128) // S
                            tloc = tm * 128 - b * S
                            ps = psum.tile([128, 512], F32, tag="bank")
                            for a in range(nq):
                                nc.tensor.matmul(
                                    ps,
                                    attnT[:, b * nq + a, ds(tloc, 128)],
                                    wo_t[:, a, :],
                                    start=(a == 0),
                                    stop=(a == nq - 1),
                                )
                            ot = ot_pool.tile([128, 512], F32)
                            if tm % 2 == 0:
                                nc.scalar.copy(ot, ps)
                            else:
                                nc.vector.tensor_copy(ot, ps)
                            nc.sync.dma_start(out[ts(tm, 128), hsl], ot)
    nc.compile()
    return nc


def prep_core_inputs(cfg: Cfg, hidden, w_qkv, w_o, core: int):
    """Build the per-core input map (all fp32, C-contiguous)."""
    T, H, S, nq, D = cfg.T, cfg.H, cfg.S, cfg.nq, cfg.D
    NQ_TOT = cfg.n_cores * nq
    xT = np.ascontiguousarray(hidden.T).reshape(cfg.KO, 128, T)
    q0 = core * nq * D
    rows = list(range(q0, q0 + nq * D))
    rows += list(range(NQ_TOT * D + core * D, NQ_TOT * D + (core + 1) * D))
    kv_heads = cfg.n_cores  # one kv head per core
    rows += list(
        range((NQ_TOT + kv_heads) * D + core * D, (NQ_TOT + kv_heads) * D + (core + 1) * D)
    )
    wqkvT = np.ascontiguousarray(w_qkv[rows, :].T).reshape(cfg.KO, 128, cfg.NM * 128)
    woT = np.ascontiguousarray(w_o[:, core * nq * D : (core + 1) * nq * D].T).reshape(
        nq, 128, H
    )
    return {"xT": xT, "wqkvT": wqkvT, "woT": woT}


def prep_shared_inputs(cfg: Cfg, rope_theta=10000.0):
    S, D = cfg.S, cfg.D
    NRT = cfg.QW // 128
    inv = 1.0 / (rope_theta ** (np.arange(0, D, 2, dtype=np.float64) / D))
    ang = np.arange(S, dtype=np.float64)[:, None] * inv[None, :]  # [S, 64]
    cos = np.cos(ang).T.astype(np.float32)  # [64, S]
    sin = np.sin(ang).T.astype(np.float32)
    cosT = np.concatenate([cos, cos], axis=0)  # [128, S]
    sinT = np.concatenate([-sin, sin], axis=0)
    kl = np.arange(128)[:, None, None]
    r = np.arange(NRT)[None, :, None]
    ql = np.arange(cfg.QW)[None, None, :]
    masksT = ((r * 128 + kl) <= ql).astype(np.float32)
    return {
        "cosT": np.ascontiguousarray(cosT),
        "sinT": np.ascontiguousarray(sinT),
        "masksT": np.ascontiguousarray(masksT),
        "identT": np.eye(128, dtype=np.float32),
        "onesT": np.ones((128, 1), dtype=np.float32),
    }


_CACHE = {}
LAST_EXEC_NS = None


def _get_nc(cfg: Cfg) -> bass.Bass:
    if cfg not in _CACHE:
        _CACHE[cfg] = build_nc(cfg)
    return _CACHE[cfg]


def kernel(hidden_states=None, w_qkv=None, w_o=None, seq_len=None, **_):
    cfg = FULL
    hidden = np.asarray(hidden_states, dtype=np.float32)
    w_qkv = np.asarray(w_qkv, dtype=np.float32)
    w_o = np.asarray(w_o, dtype=np.float32)

    nc = _get_nc(cfg)
    shared = prep_shared_inputs(cfg)
    in_maps = []
    for c in range(cfg.n_cores):
        m = dict(shared)
        m.update(prep_core_inputs(cfg, hidden, w_qkv, w_o, c))
        in_maps.append(m)

    trace = os.environ.get("KERNEL_TRACE", "0") == "1"
    res = run_bass_kernel_spmd(
        nc, in_maps, core_ids=list(range(cfg.n_cores)), trace=trace
    )
    global LAST_EXEC_NS
    if res.exec_time_ns is not None:
        LAST_EXEC_NS = res.exec_time_ns
        print(f"HW exec time: {res.exec_time_ns} ns")
        if res.instructions_and_trace is not None:
            print(f"trace: {res.instructions_and_trace[1]}")
    outs = [r["o_partial"] for r in res.results]
    acc = outs[0].astype(np.float32).copy()
    for o in outs[1:]:
        acc += o
    return acc
